# revision 9
# baseline (speedup 1.0000x reference)
"""Trainium2 Bass kernel for ConditionalHierarchicalCrossEntropyLoss.

Data-parallel: shard y_pred/y_true along batch across 8 NeuronCores;
replicate the tiny per-class table; sum the per-core partials on host.

Key identity: the softmax denominator Z cancels out of the loss.
cond = s_k/(s_next + 1e-8) with s = suffix-sums of softmax probs along
the label's tree path; both carry 1/Z, so cond = S_k/(S_next + Z*eps)
with S the *unnormalized* exp suffix sums. Z*eps ~ 1.3e-4 vs S_next >=
exp(x_root) = O(1), so dropping Z entirely changes the mean loss by
~2e-4 relative -- far inside tolerance. Hence y_pred is never read in
full: per 128-row block we read only a [128, WIN] window (all tree
nodes of depth<=3 live in columns [0, WIN)) plus 2 gathered logits
(leaf, parent) per row. That halves HBM traffic; the kernel is
y_true-argmax DMA-bound (~32 MiB/core).

Per 128-row block on each core:
  1. DMA y_true block [128, 8192] -> SBUF (SP queue). Chunked argmax
     (InstMax/InstMaxIndex have no fast DVE mode, so each full-row pass
     costs 8.5us -- one pass, not two): DVE tensor_reduce max over
     [128, 64, 128] -> [128, 64] chunk maxima; InstMax+InstMaxIndex on
     the 64 maxima pick the winning chunk; SWDGE re-gathers that 512B
     chunk from HBM; InstMaxIndex inside it gives the label
     (first-occurrence at every level, matching jnp.argmax).
  2. DMA y_pred window [128, WIN] (ACT queue); ACT exp in place.
  3. gpsimd SWDGE gathers the per-class record from a host-built DRAM
     table [C, 32]: paths, node masks [k<len], and
     wm_k = class_w * exp(-0.1*(len-1-k)) * [k < len-1].
  4. SWDGE gathers y_pred[row, path_k], k=0..1 (leaf, parent) via flat
     offsets row*8192 + path_k (one offset per partition per gather --
     the HW DGE constraint). Depth<=3 nodes (path cols 2..4) come from
     the exp'd window by iota-equality matching; col 5 is the root =
     window column 0.
  5. Tiny [128, <=8] ops: suffix sums of exp values, conditional
     ratios, ln, weighted row loss, accumulated across blocks.
Output per core: [128, 1] partials; host: loss = -sum(partials)/B.
"""

import numpy as np

import concourse.bacc as bacc
import concourse.bass as bass
import concourse.tile as tile
from concourse import mybir

N_CORES = 8
B = 8192          # batch
C = 8192          # classes
RPC = B // N_CORES  # rows per core
P = 128           # partitions / rows per block
NBLK = RPC // P   # blocks per core
D = 6             # max tree depth (padded path length)
TW = 32           # table row width (floats)
EPS = 1e-8
DEPTH_PARAM = 0.1

f32 = mybir.dt.float32
u32 = mybir.dt.uint32

Alu = mybir.AluOpType
Act = mybir.ActivationFunctionType


WIN = 585   # tree mode: nodes at depth<=3 live in columns [0, WIN)


NCHUNK = 64       # chunks per row for two-level argmax
CW = C // NCHUNK  # chunk width (128)


def _body(tc, yp_d, yt_d, tab_d, cst_d, out_d, repeats=1, tree_mode=False):
    NG = 2 if tree_mode else 6   # number of per-row random gathers
    yt64_d = yt_d.rearrange("a (b c) -> (a b) c", c=CW)
    # Software-pipelined by stage: engines execute in order, so per-block
    # chains with cross-engine round trips would stall every engine for all
    # later blocks. Emitting each stage for all 8 blocks together lets each
    # engine stream 8 homogeneous ops while SWDGE gather latencies amortize.
    nc = tc.nc
    with (
        tc.tile_pool(name="big", bufs=5) as big,
        tc.tile_pool(name="small", bufs=NBLK + 1) as small,
        tc.tile_pool(name="single", bufs=1) as single,
    ):
        cst = single.tile([P, 32], f32)
        nc.sync.dma_start(out=cst[:], in_=cst_d)
        if tree_mode:
            iota_i = single.tile([P, WIN], mybir.dt.int32)
            nc.gpsimd.iota(iota_i[:], pattern=[[1, WIN]], base=0,
                           channel_multiplier=0)
            iota_f = single.tile([P, WIN], f32)
            nc.vector.tensor_copy(out=iota_f[:], in_=iota_i[:])
            junk = single.tile([P, WIN], f32)

        acc = single.tile([P, 1], f32)

        for rep in range(repeats):
            nc.vector.memset(acc[:], 0.0)
            rowbase = [cst[:, 8 + b:9 + b] for b in range(NBLK)]
            row64 = [cst[:, 16 + b:17 + b] for b in range(NBLK)]
            st = {k: {} for k in ("m8", "cif", "offc", "chunk", "lab_u",
                                  "wc", "rec", "offu", "g",
                                  "eg", "probs", "sn", "rsn", "cond", "lc",
                                  "pl")}

            def stage_a(b):
                # big y_true load + chunk maxima + winning chunk; window+exp
                rows = slice(b * P, (b + 1) * P)
                yt = big.tile([P, C], f32, tag="yt", name=f"yt{rep}_{b}")
                nc.sync.dma_start(out=yt[:], in_=yt_d[rows, :])
                if tree_mode:
                    wc = st["wc"][b] = small.tile([P, WIN], f32, tag="wc",
                                                  name=f"wc{b}")
                    nc.scalar.dma_start(out=wc[:], in_=yp_d[rows, 0:WIN])
                    nc.scalar.activation(out=wc[:], in_=wc[:], func=Act.Exp)
                cm = small.tile([P, NCHUNK], f32, tag="cm", name=f"cm{b}")
                nc.vector.tensor_reduce(
                    out=cm[:], in_=yt[:].rearrange("p (a c) -> p a c", c=CW),
                    axis=mybir.AxisListType.X, op=Alu.max,
                )
                m8 = st["m8"][b] = small.tile([P, 8], f32, tag="m8",
                                              name=f"m8_{b}")
                nc.vector.max(m8[:], cm[:])
                ci8 = small.tile([P, 8], u32, tag="ci8", name=f"ci8_{b}")
                nc.vector.max_index(ci8[:], m8[:], cm[:])
                cif = st["cif"][b] = small.tile([P, 1], f32, tag="cif",
                                                name=f"cif{b}")
                nc.vector.tensor_copy(out=cif[:], in_=ci8[:, 0:1])
                # flat row index into the [RPC*64, 128] view of y_true
                offc = st["offc"][b] = small.tile([P, 1], u32, tag="offc",
                                                  name=f"offc{b}")
                nc.vector.tensor_scalar(
                    out=offc[:], in0=cif[:], scalar1=row64[b], scalar2=None,
                    op0=Alu.add,
                )

            def stage_b(b):
                # re-gather the winning 512B chunk from HBM
                chunk = st["chunk"][b] = small.tile([P, CW], f32, tag="chunk",
                                                    name=f"chunk{b}")
                nc.gpsimd.indirect_dma_start(
                    out=chunk[:], out_offset=None, in_=yt64_d,
                    in_offset=bass.IndirectOffsetOnAxis(
                        ap=st["offc"][b][:, 0:1], axis=0),
                )

            def stage_c(b):
                # label = chunk*128 + index-within-chunk
                inner8 = small.tile([P, 8], u32, tag="inner8",
                                    name=f"inner8_{b}")
                nc.vector.max_index(inner8[:], st["m8"][b][:],
                                    st["chunk"][b][:])
                innerf = small.tile([P, 1], f32, tag="innerf",
                                    name=f"innerf{b}")
                nc.vector.tensor_copy(out=innerf[:], in_=inner8[:, 0:1])
                lab = st["lab_u"][b] = small.tile([P, 1], u32, tag="lab_u",
                                                  name=f"lab_u{b}")
                nc.vector.scalar_tensor_tensor(
                    out=lab[:], in0=st["cif"][b][:], scalar=float(CW),
                    in1=innerf[:], op0=Alu.mult, op1=Alu.add,
                )

            def stage_d(b):
                # record gather by label
                rec = st["rec"][b] = small.tile([P, TW], f32, tag="rec",
                                                name=f"rec{b}")
                nc.gpsimd.indirect_dma_start(
                    out=rec[:], out_offset=None, in_=tab_d,
                    in_offset=bass.IndirectOffsetOnAxis(
                        ap=st["lab_u"][b][:, 0:1], axis=0),
                )

            def stage_e(b):
                # value offsets; path-logit gathers
                offu = st["offu"][b] = small.tile(
                    [P, NG], u32, tag="offu", name=f"offu{b}")
                nc.vector.tensor_scalar(
                    out=offu[:], in0=st["rec"][b][:, 0:NG],
                    scalar1=rowbase[b], scalar2=None, op0=Alu.add,
                )
                g = st["g"][b] = small.tile([P, NG], f32, tag="g",
                                            name=f"g{b}")
                for k in range(NG):
                    nc.gpsimd.indirect_dma_start(
                        out=g[:, k:k + 1], out_offset=None, in_=yp_d,
                        in_offset=bass.IndirectOffsetOnAxis(
                            ap=offu[:, k:k + 1], axis=1),
                    )

            def stage_f(b):
                # per-row loss tail (Z-free: unnormalized exp suffix sums)
                rec = st["rec"][b]
                eg = st["eg"][b] = small.tile([P, NG], f32, tag="eg",
                                              name=f"eg{b}")
                nc.scalar.activation(out=eg[:], in_=st["g"][b][:],
                                     func=Act.Exp)
                probs = st["probs"][b] = small.tile(
                    [P, D], f32, tag="probs", name=f"probs{b}")
                if tree_mode:
                    # exp'd values for depth<=3 nodes come from the SBUF
                    # window: e_k = sum_j [j == path_k] * exp(x_j)
                    wc = st["wc"][b]
                    epack = small.tile([P, 4], f32, tag="epack",
                                       name=f"epack{b}")
                    for i, (kcol, w) in enumerate(((2, WIN), (3, 73),
                                                   (4, 9))):
                        nc.vector.scalar_tensor_tensor(
                            out=junk[:, 0:w], in0=iota_f[:, 0:w],
                            scalar=rec[:, kcol:kcol + 1], in1=wc[:, 0:w],
                            op0=Alu.is_equal, op1=Alu.mult,
                            accum_out=epack[:, i:i + 1],
                        )
                    nc.vector.tensor_copy(out=epack[:, 3:4], in_=wc[:, 0:1])
                    nc.vector.tensor_tensor(
                        out=probs[:, 0:2], in0=eg[:], in1=rec[:, 16:18],
                        op=Alu.mult,
                    )
                    nc.vector.tensor_tensor(
                        out=probs[:, 2:6], in0=epack[:], in1=rec[:, 18:22],
                        op=Alu.mult,
                    )
                else:
                    nc.vector.tensor_tensor(
                        out=probs[:], in0=eg[:], in1=rec[:, 16:22],
                        op=Alu.mult,
                    )
                for k in range(D - 2, -1, -1):
                    nc.scalar.add(probs[:, k:k + 1], probs[:, k:k + 1],
                                  probs[:, k + 1:k + 2])
                sn = st["sn"][b] = small.tile([P, D - 1], f32, tag="sn",
                                              name=f"sn{b}")
                nc.scalar.activation(out=sn[:], in_=probs[:, 1:6],
                                     func=Act.Identity, bias=cst[:, 7:8])
                rsn = st["rsn"][b] = small.tile([P, D - 1], f32, tag="rsn",
                                                name=f"rsn{b}")
                nc.vector.reciprocal(rsn[:], sn[:])
                cond = st["cond"][b] = small.tile(
                    [P, D - 1], f32, tag="cond", name=f"cond{b}")
                nc.vector.tensor_tensor(out=cond[:], in0=probs[:, 0:5],
                                        in1=rsn[:], op=Alu.mult)
                lc = st["lc"][b] = small.tile([P, D - 1], f32, tag="lc",
                                              name=f"lc{b}")
                nc.scalar.activation(out=lc[:], in_=cond[:], func=Act.Ln,
                                     bias=cst[:, 7:8])
                t2 = small.tile([P, D - 1], f32, tag="t2", name=f"t2_{b}")
                pl = st["pl"][b] = small.tile([P, 1], f32, tag="pl",
                                              name=f"pl{b}")
                nc.vector.scalar_tensor_tensor(
                    out=t2[:], in0=lc[:], scalar=1.0, in1=rec[:, 8:13],
                    op0=Alu.mult, op1=Alu.mult, accum_out=pl[:],
                )
                nc.vector.tensor_tensor(out=acc[:], in0=acc[:], in1=pl[:],
                                        op=Alu.add)

            # software pipeline with block lag so each engine's in-order
            # stream interleaves stages of different blocks
            stages = (stage_a, stage_b, stage_c, stage_d, stage_e, stage_f)
            for s in range(NBLK + len(stages) - 1):
                for i, stg in enumerate(stages):
                    if 0 <= s - i < NBLK:
                        stg(s - i)

        nc.sync.dma_start(out=out_d, in_=acc[:])


def build_bass(repeats=1, tree_mode=False):
    nc = bacc.Bacc("TRN2", target_bir_lowering=False, debug=False,
                   enable_asserts=False)
    yp = nc.dram_tensor("y_pred_s", [RPC, C], f32, kind="ExternalInput")
    yt = nc.dram_tensor("y_true_s", [RPC, C], f32, kind="ExternalInput")
    tab = nc.dram_tensor("table", [C, TW], f32, kind="ExternalInput")
    cst = nc.dram_tensor("consts", [P, 32], f32, kind="ExternalInput")
    out = nc.dram_tensor("partial", [P, 1], f32, kind="ExternalOutput")
    with tile.TileContext(nc) as tc:
        _body(tc, yp.ap(), yt.ap(), tab.ap(), cst.ap(), out.ap(),
              repeats=repeats, tree_mode=tree_mode)
    nc.compile()
    return nc


def make_host_tables(class_w, tree_paths, tree_lens):
    class_w = np.asarray(class_w, np.float64)
    lens = np.asarray(tree_lens, np.float64)
    table = np.zeros((C, TW), np.float32)
    table[:, 0:6] = np.asarray(tree_paths, np.float32)[:, 0:6]
    table[:, 6] = lens.astype(np.float32)
    k5 = np.arange(D - 1, dtype=np.float64)
    h = lens[:, None] - 1.0 - k5[None, :]
    w = np.exp(-DEPTH_PARAM * h.astype(np.float32).astype(np.float64))
    valid = k5[None, :] < (lens[:, None] - 1.0)
    table[:, 8:13] = (class_w[:, None] * w * valid).astype(np.float32)
    k6 = np.arange(D, dtype=np.float64)
    table[:, 16:22] = (k6[None, :] < lens[:, None]).astype(np.float32)

    consts = np.zeros((P, 32), np.float32)
    consts[:, 0:6] = np.arange(D, dtype=np.float32)[None, :]
    consts[:, 6] = 1.0
    consts[:, 7] = EPS
    p_idx = np.arange(P, dtype=np.float32)
    for b in range(NBLK):
        consts[:, 8 + b] = (b * P + p_idx) * C
        consts[:, 16 + b] = (b * P + p_idx) * NCHUNK
    return table, consts


def make_in_maps(y_pred, y_true, table, consts):
    y_pred = np.ascontiguousarray(np.asarray(y_pred, np.float32))
    y_true = np.ascontiguousarray(np.asarray(y_true, np.float32))
    in_maps = []
    for c in range(N_CORES):
        in_maps.append({
            "y_pred_s": y_pred[c * RPC:(c + 1) * RPC],
            "y_true_s": y_true[c * RPC:(c + 1) * RPC],
            "table": table,
            "consts": consts,
        })
    return in_maps


_NC = {}


def tree_bounds_ok(tree_paths):
    p = np.asarray(tree_paths)
    return bool((p[:, 2].max() < WIN) and (p[:, 3].max() < 73)
                and (p[:, 4].max() < 9))


def kernel(y_pred, y_true, class_w, tree_paths, tree_lens):
    from concourse.bass_utils import run_bass_kernel_spmd
    tm = tree_bounds_ok(tree_paths)
    if tm not in _NC:
        _NC[tm] = build_bass(tree_mode=tm)
    _nc = _NC[tm]
    table, consts = make_host_tables(class_w, tree_paths, tree_lens)
    in_maps = make_in_maps(y_pred, y_true, table, consts)
    res = run_bass_kernel_spmd(_nc, in_maps, core_ids=list(range(N_CORES)))
    total = sum(float(r["partial"].sum()) for r in res.results)
    return np.float32(-total / B)


if __name__ == "__main__":
    nc = build_bass(tree_mode=True)
    print("built OK:", len(nc.m.functions[0].allocations), "allocations")


# revision 12
# speedup vs baseline: 1.2600x; 1.2600x over previous
"""Trainium2 Bass kernel for ConditionalHierarchicalCrossEntropyLoss.

Data-parallel: shard y_pred/y_true along batch across 8 NeuronCores;
replicate the tiny per-class table; sum the per-core partials on host.

Key identity: the softmax denominator Z cancels out of the loss.
cond = s_k/(s_next + 1e-8) with s = suffix-sums of softmax probs along
the label's tree path; both carry 1/Z, so cond = S_k/(S_next + Z*eps)
with S the *unnormalized* exp suffix sums. Z*eps ~ 1.3e-4 vs S_next >=
exp(x_root) = O(1), so dropping Z entirely changes the mean loss by
~2e-4 relative -- far inside tolerance. Hence y_pred is never read in
full: per 128-row block we read only a [128, WIN] window (all tree
nodes of depth<=3 live in columns [0, WIN)) plus 2 gathered logits
(leaf, parent) per row. That halves HBM traffic; the kernel is
y_true-argmax DMA-bound (~32 MiB/core).

Per 128-row block on each core:
  1. DMA y_true block [128, 8192] -> SBUF (SP queue). Chunked argmax
     (InstMax/InstMaxIndex have no fast DVE mode, so each full-row pass
     costs 8.5us -- one pass, not two): DVE tensor_reduce max over
     [128, 64, 128] -> [128, 64] chunk maxima; InstMax+InstMaxIndex on
     the 64 maxima pick the winning chunk; SWDGE re-gathers that 512B
     chunk from HBM; InstMaxIndex inside it gives the label
     (first-occurrence at every level, matching jnp.argmax).
  2. DMA y_pred window [128, WIN] (ACT queue); ACT exp in place.
  3. gpsimd SWDGE gathers the per-class record from a host-built DRAM
     table [C, 32]: paths, node masks [k<len], and
     wm_k = class_w * exp(-0.1*(len-1-k)) * [k < len-1].
  4. SWDGE gathers y_pred[row, path_k], k=0..1 (leaf, parent) via flat
     offsets row*8192 + path_k (one offset per partition per gather --
     the HW DGE constraint). Depth<=3 nodes (path cols 2..4) come from
     the exp'd window by iota-equality matching; col 5 is the root =
     window column 0.
  5. Tiny [128, <=8] ops: suffix sums of exp values, conditional
     ratios, ln, weighted row loss, accumulated across blocks.
Output per core: [128, 1] partials; host: loss = -sum(partials)/B.
"""

import numpy as np

import concourse.bacc as bacc
import concourse.bass as bass
import concourse.tile as tile
from concourse import mybir

N_CORES = 8
B = 8192          # batch
C = 8192          # classes
RPC = B // N_CORES  # rows per core
P = 128           # partitions / rows per block
NBLK = RPC // P   # blocks per core
D = 6             # max tree depth (padded path length)
TW = 32           # table row width (floats)
EPS = 1e-8
DEPTH_PARAM = 0.1

f32 = mybir.dt.float32
u32 = mybir.dt.uint32

Alu = mybir.AluOpType
Act = mybir.ActivationFunctionType


WIN = 585   # tree mode: nodes at depth<=3 live in columns [0, WIN)


NCHUNK = 64       # chunks per row for two-level argmax
CW = C // NCHUNK  # chunk width (128)


def _body(tc, yp_d, yt_d, tab_d, cst_d, out_d, repeats=1, tree_mode=False):
    NG = 2 if tree_mode else 6   # number of per-row random gathers
    yt64_d = yt_d.rearrange("a (b c) -> (a b) c", c=CW)
    # Software-pipelined by stage: engines execute in order, so per-block
    # chains with cross-engine round trips would stall every engine for all
    # later blocks. Emitting each stage for all 8 blocks together lets each
    # engine stream 8 homogeneous ops while SWDGE gather latencies amortize.
    nc = tc.nc
    with (
        tc.tile_pool(name="big", bufs=4) as big,
        tc.tile_pool(name="small", bufs=NBLK + 4) as small,
        tc.tile_pool(name="single", bufs=1) as single,
    ):
        cst = single.tile([P, 32], f32)
        nc.sync.dma_start(out=cst[:], in_=cst_d)
        if tree_mode:
            iota_i = single.tile([P, WIN], mybir.dt.int32)
            nc.gpsimd.iota(iota_i[:], pattern=[[1, WIN]], base=0,
                           channel_multiplier=0)
            iota_f = single.tile([P, WIN], f32)
            nc.vector.tensor_copy(out=iota_f[:], in_=iota_i[:])
            junk = single.tile([P, WIN], f32)

        acc = single.tile([P, 1], f32)

        for rep in range(repeats):
            nc.vector.memset(acc[:], 0.0)
            rowbase = [cst[:, 8 + b:9 + b] for b in range(NBLK)]
            row64 = [cst[:, 16 + b:17 + b] for b in range(NBLK)]
            st = {k: {} for k in ("m8", "cif", "offc", "chunk", "lab_u",
                                  "wc", "rec", "offu", "g",
                                  "eg", "probs", "sn", "rsn", "cond", "lc",
                                  "pl")}

            def stage_a(b):
                # big y_true load + chunk maxima + winning chunk; window+exp
                rows = slice(b * P, (b + 1) * P)
                yt = big.tile([P, C], f32, tag="yt", name=f"yt{rep}_{b}")
                # alternate HWDGE queues (qSPDynamicHW / qActDynamicHW) so
                # both issue paths keep the DMA engines fed
                dma_eng = nc.sync if b % 2 == 0 else nc.scalar
                dma_eng.dma_start(out=yt[:], in_=yt_d[rows, :])
                if tree_mode:
                    wc = st["wc"][b] = small.tile([P, WIN], f32, tag="wc",
                                                  name=f"wc{b}")
                    nc.scalar.dma_start(out=wc[:], in_=yp_d[rows, 0:WIN])
                    nc.scalar.activation(out=wc[:], in_=wc[:], func=Act.Exp)
                cm = small.tile([P, NCHUNK], f32, tag="cm", name=f"cm{b}")
                nc.vector.tensor_reduce(
                    out=cm[:], in_=yt[:].rearrange("p (a c) -> p a c", c=CW),
                    axis=mybir.AxisListType.X, op=Alu.max,
                )
                m8 = st["m8"][b] = small.tile([P, 8], f32, tag="m8",
                                              name=f"m8_{b}")
                nc.vector.max(m8[:], cm[:])
                ci8 = small.tile([P, 8], u32, tag="ci8", name=f"ci8_{b}")
                nc.vector.max_index(ci8[:], m8[:], cm[:])
                cif = st["cif"][b] = small.tile([P, 1], f32, tag="cif",
                                                name=f"cif{b}")
                nc.vector.tensor_copy(out=cif[:], in_=ci8[:, 0:1])
                # flat row index into the [RPC*64, 128] view of y_true
                offc = st["offc"][b] = small.tile([P, 1], u32, tag="offc",
                                                  name=f"offc{b}")
                nc.vector.tensor_scalar(
                    out=offc[:], in0=cif[:], scalar1=row64[b], scalar2=None,
                    op0=Alu.add,
                )

            def stage_b(b):
                # re-gather the winning 512B chunk from HBM
                chunk = st["chunk"][b] = small.tile([P, CW], f32, tag="chunk",
                                                    name=f"chunk{b}")
                nc.gpsimd.indirect_dma_start(
                    out=chunk[:], out_offset=None, in_=yt64_d,
                    in_offset=bass.IndirectOffsetOnAxis(
                        ap=st["offc"][b][:, 0:1], axis=0),
                )

            def stage_c(b):
                # label = chunk*128 + index-within-chunk
                inner8 = small.tile([P, 8], u32, tag="inner8",
                                    name=f"inner8_{b}")
                nc.vector.max_index(inner8[:], st["m8"][b][:],
                                    st["chunk"][b][:])
                innerf = small.tile([P, 1], f32, tag="innerf",
                                    name=f"innerf{b}")
                nc.vector.tensor_copy(out=innerf[:], in_=inner8[:, 0:1])
                lab = st["lab_u"][b] = small.tile([P, 1], u32, tag="lab_u",
                                                  name=f"lab_u{b}")
                nc.vector.scalar_tensor_tensor(
                    out=lab[:], in0=st["cif"][b][:], scalar=float(CW),
                    in1=innerf[:], op0=Alu.mult, op1=Alu.add,
                )

            def stage_d(b):
                # record gather by label
                rec = st["rec"][b] = small.tile([P, TW], f32, tag="rec",
                                                name=f"rec{b}")
                nc.gpsimd.indirect_dma_start(
                    out=rec[:], out_offset=None, in_=tab_d,
                    in_offset=bass.IndirectOffsetOnAxis(
                        ap=st["lab_u"][b][:, 0:1], axis=0),
                )

            def stage_e(b):
                # value offsets; path-logit gathers
                offu = st["offu"][b] = small.tile(
                    [P, NG], u32, tag="offu", name=f"offu{b}")
                nc.vector.tensor_scalar(
                    out=offu[:], in0=st["rec"][b][:, 0:NG],
                    scalar1=rowbase[b], scalar2=None, op0=Alu.add,
                )
                g = st["g"][b] = small.tile([P, NG], f32, tag="g",
                                            name=f"g{b}")
                for k in range(NG):
                    nc.gpsimd.indirect_dma_start(
                        out=g[:, k:k + 1], out_offset=None, in_=yp_d,
                        in_offset=bass.IndirectOffsetOnAxis(
                            ap=offu[:, k:k + 1], axis=1),
                    )

            def stage_f(b):
                # per-row loss tail (Z-free: unnormalized exp suffix sums)
                rec = st["rec"][b]
                eg = st["eg"][b] = small.tile([P, NG], f32, tag="eg",
                                              name=f"eg{b}")
                nc.scalar.activation(out=eg[:], in_=st["g"][b][:],
                                     func=Act.Exp)
                probs = st["probs"][b] = small.tile(
                    [P, D], f32, tag="probs", name=f"probs{b}")
                if tree_mode:
                    # exp'd values for depth<=3 nodes come from the SBUF
                    # window: e_k = sum_j [j == path_k] * exp(x_j)
                    wc = st["wc"][b]
                    epack = small.tile([P, 4], f32, tag="epack",
                                       name=f"epack{b}")
                    for i, (kcol, w) in enumerate(((2, WIN), (3, 73),
                                                   (4, 9))):
                        nc.vector.scalar_tensor_tensor(
                            out=junk[:, 0:w], in0=iota_f[:, 0:w],
                            scalar=rec[:, kcol:kcol + 1], in1=wc[:, 0:w],
                            op0=Alu.is_equal, op1=Alu.mult,
                            accum_out=epack[:, i:i + 1],
                        )
                    nc.vector.tensor_copy(out=epack[:, 3:4], in_=wc[:, 0:1])
                    nc.vector.tensor_tensor(
                        out=probs[:, 0:2], in0=eg[:], in1=rec[:, 16:18],
                        op=Alu.mult,
                    )
                    nc.vector.tensor_tensor(
                        out=probs[:, 2:6], in0=epack[:], in1=rec[:, 18:22],
                        op=Alu.mult,
                    )
                else:
                    nc.vector.tensor_tensor(
                        out=probs[:], in0=eg[:], in1=rec[:, 16:22],
                        op=Alu.mult,
                    )
                for k in range(D - 2, -1, -1):
                    nc.scalar.add(probs[:, k:k + 1], probs[:, k:k + 1],
                                  probs[:, k + 1:k + 2])
                sn = st["sn"][b] = small.tile([P, D - 1], f32, tag="sn",
                                              name=f"sn{b}")
                nc.scalar.activation(out=sn[:], in_=probs[:, 1:6],
                                     func=Act.Identity, bias=cst[:, 7:8])
                rsn = st["rsn"][b] = small.tile([P, D - 1], f32, tag="rsn",
                                                name=f"rsn{b}")
                nc.vector.reciprocal(rsn[:], sn[:])
                cond = st["cond"][b] = small.tile(
                    [P, D - 1], f32, tag="cond", name=f"cond{b}")
                nc.vector.tensor_tensor(out=cond[:], in0=probs[:, 0:5],
                                        in1=rsn[:], op=Alu.mult)
                lc = st["lc"][b] = small.tile([P, D - 1], f32, tag="lc",
                                              name=f"lc{b}")
                nc.scalar.activation(out=lc[:], in_=cond[:], func=Act.Ln,
                                     bias=cst[:, 7:8])
                t2 = small.tile([P, D - 1], f32, tag="t2", name=f"t2_{b}")
                pl = st["pl"][b] = small.tile([P, 1], f32, tag="pl",
                                              name=f"pl{b}")
                nc.vector.scalar_tensor_tensor(
                    out=t2[:], in0=lc[:], scalar=1.0, in1=rec[:, 8:13],
                    op0=Alu.mult, op1=Alu.mult, accum_out=pl[:],
                )
                nc.vector.tensor_tensor(out=acc[:], in0=acc[:], in1=pl[:],
                                        op=Alu.add)

            # Software pipeline with per-stage lags. Engines execute their
            # queues in order, so a stage that consumes another engine's
            # output must be scheduled >= 2 pipeline periods after the
            # producer: an SWDGE gather's full round trip (desc-gen on Q7 ->
            # DMA engines -> completion semaphore) is comparable to a whole
            # period, and a consumer reaching the head of an in-order queue
            # too early stalls every later op on that engine (head-of-line
            # blocking). Lags: chunk gather 1 after its offsets, each
            # SWDGE->engine hop +2.
            stages = ((0, stage_a), (1, stage_b), (3, stage_c), (5, stage_d),
                      (7, stage_e), (9, stage_f))
            for s in range(NBLK + stages[-1][0]):
                for lag, stg in stages:
                    if 0 <= s - lag < NBLK:
                        stg(s - lag)

        nc.sync.dma_start(out=out_d, in_=acc[:])


def build_bass(repeats=1, tree_mode=False):
    nc = bacc.Bacc("TRN2", target_bir_lowering=False, debug=False,
                   enable_asserts=False)
    yp = nc.dram_tensor("y_pred_s", [RPC, C], f32, kind="ExternalInput")
    yt = nc.dram_tensor("y_true_s", [RPC, C], f32, kind="ExternalInput")
    tab = nc.dram_tensor("table", [C, TW], f32, kind="ExternalInput")
    cst = nc.dram_tensor("consts", [P, 32], f32, kind="ExternalInput")
    out = nc.dram_tensor("partial", [P, 1], f32, kind="ExternalOutput")
    with tile.TileContext(nc) as tc:
        _body(tc, yp.ap(), yt.ap(), tab.ap(), cst.ap(), out.ap(),
              repeats=repeats, tree_mode=tree_mode)
    nc.compile()
    return nc


def make_host_tables(class_w, tree_paths, tree_lens):
    class_w = np.asarray(class_w, np.float64)
    lens = np.asarray(tree_lens, np.float64)
    table = np.zeros((C, TW), np.float32)
    table[:, 0:6] = np.asarray(tree_paths, np.float32)[:, 0:6]
    table[:, 6] = lens.astype(np.float32)
    k5 = np.arange(D - 1, dtype=np.float64)
    h = lens[:, None] - 1.0 - k5[None, :]
    w = np.exp(-DEPTH_PARAM * h.astype(np.float32).astype(np.float64))
    valid = k5[None, :] < (lens[:, None] - 1.0)
    table[:, 8:13] = (class_w[:, None] * w * valid).astype(np.float32)
    k6 = np.arange(D, dtype=np.float64)
    table[:, 16:22] = (k6[None, :] < lens[:, None]).astype(np.float32)

    consts = np.zeros((P, 32), np.float32)
    consts[:, 0:6] = np.arange(D, dtype=np.float32)[None, :]
    consts[:, 6] = 1.0
    consts[:, 7] = EPS
    p_idx = np.arange(P, dtype=np.float32)
    for b in range(NBLK):
        consts[:, 8 + b] = (b * P + p_idx) * C
        consts[:, 16 + b] = (b * P + p_idx) * NCHUNK
    return table, consts


def make_in_maps(y_pred, y_true, table, consts):
    y_pred = np.ascontiguousarray(np.asarray(y_pred, np.float32))
    y_true = np.ascontiguousarray(np.asarray(y_true, np.float32))
    in_maps = []
    for c in range(N_CORES):
        in_maps.append({
            "y_pred_s": y_pred[c * RPC:(c + 1) * RPC],
            "y_true_s": y_true[c * RPC:(c + 1) * RPC],
            "table": table,
            "consts": consts,
        })
    return in_maps


_NC = {}


def tree_bounds_ok(tree_paths):
    p = np.asarray(tree_paths)
    return bool((p[:, 2].max() < WIN) and (p[:, 3].max() < 73)
                and (p[:, 4].max() < 9))


def kernel(y_pred, y_true, class_w, tree_paths, tree_lens):
    from concourse.bass_utils import run_bass_kernel_spmd
    tm = tree_bounds_ok(tree_paths)
    if tm not in _NC:
        _NC[tm] = build_bass(tree_mode=tm)
    _nc = _NC[tm]
    table, consts = make_host_tables(class_w, tree_paths, tree_lens)
    in_maps = make_in_maps(y_pred, y_true, table, consts)
    res = run_bass_kernel_spmd(_nc, in_maps, core_ids=list(range(N_CORES)))
    total = sum(float(r["partial"].sum()) for r in res.results)
    return np.float32(-total / B)


if __name__ == "__main__":
    nc = build_bass(tree_mode=True)
    print("built OK:", len(nc.m.functions[0].allocations), "allocations")
